# revision 13
# baseline (speedup 1.0000x reference)
"""Trainium2 Bass kernel for nn_CAMLocalHead (CAM target + conv head + BCE).

Self-contained: takes FULL inputs, shards batch B=8 across 8 NeuronCores
(one sample per core), runs a Bass/Tile kernel per core, sums the per-core
partial BCE sums on host.

Device algorithm per core (one sample):
  - argmax class via one-hot (sigmoid is monotonic), selected proj row via
    PE matmuls, CAM = row @ x as fp8 DoubleRow matmuls (scale-invariant).
  - top-392-of-784 mask via rank trick: rank(v) = #{j: cam_j >= v} <= 392,
    computed with a PE broadcast + DVE is_ge accumulations (no sort).
  - Conv3d(2048->512, 1x3x3, pad 011) as 9 shifted fp8 DoubleRow matmuls
    accumulating in PSUM; x stored as 3 w-shifted padded copies so each
    tap reads contiguous 49-element runs per t-plane (no junk columns).
    Weights pre-scaled x64 into e4m3 range; un-scaled via ReLU activation
    scale=1/64. ReLU+bias fused on ACT; score conv = one more matmul per
    d-tile accumulating into a [1, 392] psum.
  - BCE sum = sum ln(1+e^x) - sum x*y  (softplus via Exp then Ln(1+e)).
"""
import sys

for _p in ("/opt/trn_rl_repo", "/opt/pypackages"):
    if _p not in sys.path:
        sys.path.append(_p)

import numpy as np
import ml_dtypes

# Problem dims (hardcoded per spec)
B, C, T, H, W = 8, 2048, 16, 7, 7
K, D = 400, 512
N_TOKEN = 392
P = 128
CT = C // P          # 16 c-tiles
CTP = CT // 2        # 8 c-tile pairs (DoubleRow)
DT = D // P          # 4 d-tiles
NH = 2               # spatial halves (t 0..7, 8..15)
TH = T // NH         # 8
NF = TH * H * W      # 392 positions per half
NPOS = T * H * W     # 784
PADN = 7 * P         # 896 (784 padded to 7 chunks of 128)
NEG = -1.0e30
SHW = 9 * 7          # 63: one w-shifted padded plane (9 rows x 7 cols)
SPT = T * SHW        # 1008: one shift-copy, all t
XF = 2 * 3 * SPT     # 6048: free size of one fp8 x pair-tile

_cache = {}


def _build_nc():
    import concourse.bacc as bacc
    import concourse.mybir as mybir
    from concourse import tile
    from concourse.tile_rust import add_dep_helper

    f32 = mybir.dt.float32
    bf16 = mybir.dt.bfloat16
    fp8 = mybir.dt.float8e4
    DR = mybir.MatmulPerfMode.DoubleRow
    AX = mybir.AxisListType.X
    OP = mybir.AluOpType
    AF = mybir.ActivationFunctionType

    nc = bacc.Bacc(trn_type="TRN2")

    w8_d = nc.dram_tensor("w8", [DT, P, CTP * 9 * 2 * P], fp8,
                          kind="ExternalInput")
    xp8_d = nc.dram_tensor("xp8", [CTP, P, XF], fp8, kind="ExternalInput")
    proj_d = nc.dram_tensor("proj", [K, C], bf16, kind="ExternalInput")
    xfp_d = nc.dram_tensor("xfp", [1, K], f32, kind="ExternalInput")
    cb_d = nc.dram_tensor("cb", [P, DT], f32, kind="ExternalInput")
    sw_d = nc.dram_tensor("sw", [P, DT], bf16, kind="ExternalInput")
    sb_d = nc.dram_tensor("sb", [1, 1], f32, kind="ExternalInput")
    out_d = nc.dram_tensor("out", [1, 1], f32, kind="ExternalOutput")

    with tile.TileContext(nc) as tc:
        with (
            tc.tile_pool(name="const", bufs=1) as cp,
            tc.tile_pool(name="wps_", bufs=4) as wp,
            tc.tile_pool(name="wpb_", bufs=2) as wpb,
            tc.tile_pool(name="rp", bufs=4) as rp,
            tc.tile_pool(name="cps", bufs=2, space="PSUM") as cps,
            tc.tile_pool(name="sps", bufs=1, space="PSUM") as sps,
            tc.tile_pool(name="mps", bufs=2, space="PSUM") as mps,
        ):
            # ---------- small constants (scalar HWDGE ring) ----------
            xfp = cp.tile([1, K], f32)
            nc.scalar.dma_start(xfp[:], xfp_d[:])
            proj_sb = cp.tile([P, 4 * C], bf16)
            for kc in range(4):
                kcnt = min(P, K - kc * P)
                nc.scalar.dma_start(
                    proj_sb[0:kcnt, kc * C:(kc + 1) * C],
                    proj_d[kc * P:kc * P + kcnt, :])
            cb_sb = cp.tile([P, DT], f32)
            nc.scalar.dma_start(cb_sb[:], cb_d[:])
            sw_sb = cp.tile([P, DT], bf16)
            nc.scalar.dma_start(sw_sb[:], sw_d[:])
            sb_sb = cp.tile([1, 1], f32)
            nc.scalar.dma_start(sb_sb[:], sb_d[:])

            ones11 = cp.tile([1, 1], f32)
            nc.vector.memset(ones11[:], 1.0)
            warm = cp.tile([1, 1], f32)
            nc.scalar.activation(warm[:], ones11[:], AF.Exp)
            nc.scalar.activation(warm[:], ones11[:], AF.Ln, bias=1.0)
            ones_row = cp.tile([1, P], f32)
            nc.vector.memset(ones_row[:], 1.0)
            ones_col = cp.tile([P, 1], f32)
            nc.vector.memset(ones_col[:], 1.0)

            xp8tiles = [cp.tile([P, XF], fp8, name=f"xp8_{i}")
                        for i in range(CTP)]

            def xp8view(ctp):
                # [p, two, s, t, f63]
                return xp8tiles[ctp][:].rearrange(
                    "p (two s t f) -> p two s t f", two=2, s=3, t=T, f=SHW)

            def conv_rhs(ctp, tap, nh):
                dh, dw = tap // 3, tap % 3
                v = xp8view(ctp)[:, :, dw, nh * TH:(nh + 1) * TH,
                                 dh * 7:dh * 7 + 49]
                return v  # [p, 2, TH, 49] -> free 784, halved by DoubleRow

            # ---------- CAM front-end (emitted between conv dt1 and dt2
            # so its DMA/DVE deps resolve while PE chews on conv) --------
            fe = {}

            def emit_frontend():
                # argmax class via one-hot (sigmoid monotonic)
                m = cp.tile([1, 1], f32)
                nc.vector.reduce_max(m[:], xfp[:], axis=AX)
                oh = cp.tile([1, 4 * P], f32)
                nc.vector.memset(oh[:], 0.0)
                nc.vector.tensor_scalar(oh[0:1, 0:K], xfp[:], m[:], None,
                                        op0=OP.is_equal)
                ohT_ps = mps.tile([P, 4], f32, tag="mp")
                for i in range(4):
                    tr = nc.tensor.transpose(ohT_ps[:, i:i + 1],
                                             oh[0:1, i * P:(i + 1) * P],
                                             ones11[:])
                    if fe.get("gate") is not None:
                        add_dep_helper(tr.ins, fe["gate"].ins, False,
                                       "defer front-end past conv dt0")
                ohT = cp.tile([P, 4], bf16)
                nc.vector.tensor_copy(ohT[:], ohT_ps[:])

                # w_selT[c] = proj_weight[top_cls, c], [128, CT] c-tile cols
                wsel_ps = mps.tile([P, CT], f32, tag="mp")
                for ct in range(CT):
                    for kc in range(4):
                        kcnt = min(P, K - kc * P)
                        nc.tensor.matmul(
                            wsel_ps[:, ct:ct + 1],
                            proj_sb[0:kcnt,
                                    kc * C + ct * P:kc * C + (ct + 1) * P],
                            ohT[0:kcnt, kc:kc + 1],
                            start=(kc == 0), stop=(kc == 3))
                # wsel8[p, two*16 + ctp] = 64 * wsel[p, 2*ctp+two], fp8
                wsel8 = cp.tile([P, 32], fp8)
                wv_out = wsel8[:].rearrange("p (two q) -> p two q", two=2)
                wv_in = wsel_ps[:].rearrange("p (q two) -> p two q", two=2)
                nc.vector.tensor_scalar(wv_out[:, :, 0:CTP], wv_in, 64.0,
                                        None, op0=OP.mult)

                # cam[1, 784] = w_sel @ x (center tap), fp8 DoubleRow
                cam_ps = [mps.tile([1, NF], f32, tag="mp", name=f"cam_ps{_h}")
                          for _h in range(NH)]
                for nh in range(NH):
                    for ctp in range(CTP):
                        nc.tensor.matmul(
                            cam_ps[nh][:],
                            wv_out[:, :, ctp:ctp + 1],
                            conv_rhs(ctp, 4, nh),
                            start=(ctp == 0), stop=(ctp == CTP - 1),
                            perf_mode=DR)
                cam_row = cp.tile([1, PADN], f32)
                nc.vector.memset(cam_row[:], NEG)
                for nh in range(NH):
                    nc.vector.tensor_copy(
                        cam_row[0:1, nh * NF:(nh + 1) * NF], cam_ps[nh][:])

                # min/max for the (monotonic) normalization, done off the
                # PE critical path; ranks use RAW cam values.
                cmin = cp.tile([1, 1], f32)
                cmax = cp.tile([1, 1], f32)
                nc.vector.tensor_reduce(cmin[:], cam_row[0:1, 0:NPOS],
                                        axis=AX, op=OP.min)
                nc.vector.reduce_max(cmax[:], cam_row[0:1, 0:NPOS], axis=AX)
                rng_t = cp.tile([1, 1], f32)
                nc.vector.tensor_scalar(rng_t[:], cmax[:], cmin[:], None,
                                        op0=OP.subtract)
                inv = cp.tile([1, 1], f32)
                nc.vector.reciprocal(inv[:], rng_t[:])

                # broadcast raw cam across partitions: camB[128, 784]
                camB = cp.tile([P, NPOS], f32)
                for nh in range(NH):
                    cb_ps = mps.tile([P, NF], f32, tag="mp")
                    nc.tensor.matmul(cb_ps[:], ones_row[:],
                                     cam_row[0:1, nh * NF:(nh + 1) * NF],
                                     start=True, stop=True)
                    nc.vector.tensor_copy(
                        camB[:, nh * NF:(nh + 1) * NF], cb_ps[:])

                # raw cam in partition layout [128, 7]
                cnp_ps = mps.tile([P, 7], f32, tag="mp")
                for a in range(7):
                    nc.tensor.transpose(cnp_ps[:, a:a + 1],
                                        cam_row[0:1, a * P:(a + 1) * P],
                                        ones11[:])
                camP = cp.tile([P, 7], f32)
                nc.vector.tensor_copy(camP[:], cnp_ps[:])

                # rank[p,a] = #{j: cam[j] >= cam[p,a]}; top-392 = rank<=392
                ge = cp.tile([P, NPOS], f32)
                rank = cp.tile([P, 7], f32)
                for a in range(7):
                    nc.vector.tensor_scalar(ge[:], camB[:],
                                            camP[:, a:a + 1],
                                            None, op0=OP.is_ge, op1=OP.add,
                                            accum_out=rank[:, a:a + 1])
                maskP = cp.tile([P, 7], f32)
                nc.vector.tensor_scalar(maskP[:], rank[:], float(N_TOKEN),
                                        None, op0=OP.is_le)
                # y = mask * (cam - cmin) * inv; keep mask*cam (raw) and
                # mask separately -- min-max norm is folded into the final
                # scalar combine: sum(y*x) = inv*(S1 - cmin*S2).
                ymP = cp.tile([P, 7], f32)
                nc.vector.tensor_mul(ymP[:], maskP[:], camP[:])
                fe["ymP"] = ymP
                fe["maskP"] = maskP
                fe["cmin"] = cmin
                fe["inv"] = inv

            # ---------- conv main loop (fp8 DoubleRow) ----------
            s_ps = [sps.tile([1, NF], f32, tag=f"s{nh}", name=f"s_ps{nh}")
                    for nh in range(NH)]

            def emit_conv_dt(dt):
                ps = [cps.tile([P, NF], f32, tag=f"cv{nh}",
                               name=f"ps{dt}_{nh}")
                      for nh in range(NH)]
                if dt == 0:
                    wtile = None
                else:
                    wtile = wpb.tile([P, CTP * 9 * 2 * P], fp8, name="w_big",
                                     tag="w_big")
                    nc.sync.dma_start(wtile[:], w8_d[dt])
                for ctp in range(CTP):
                    if dt == 0:
                        w_ct = wp.tile([P, 9 * 2 * P], fp8, name="w_ct",
                                       tag="w_ct")
                        nc.sync.dma_start(
                            w_ct[:],
                            w8_d[dt][:, ctp * 9 * 2 * P:
                                     (ctp + 1) * 9 * 2 * P])
                        nc.sync.dma_start(xp8tiles[ctp][:], xp8_d[ctp])
                    for tap in range(9):
                        if dt == 0:
                            wsl = w_ct[:, tap * 2 * P:(tap + 1) * 2 * P]
                        else:
                            wsl = wtile[:, (ctp * 9 + tap) * 2 * P:
                                        (ctp * 9 + tap + 1) * 2 * P]
                        lhsT3 = wsl.rearrange("p (two q) -> p two q", two=2)
                        for nh in range(NH):
                            nc.tensor.matmul(
                                ps[nh][:], lhsT3, conv_rhs(ctp, tap, nh),
                                start=(ctp == 0 and tap == 0),
                                stop=(ctp == CTP - 1 and tap == 8),
                                perf_mode=DR)
                last = None
                for nh in range(NH):
                    relu_t = rp.tile([P, NF], bf16, name="relu_t")
                    nc.scalar.activation(relu_t[:], ps[nh][:], AF.Relu,
                                         bias=cb_sb[:, dt:dt + 1],
                                         scale=1.0 / 64.0)
                    last = nc.tensor.matmul(s_ps[nh][:], sw_sb[:, dt:dt + 1],
                                            relu_t[:],
                                            start=(dt == 0),
                                            stop=(dt == DT - 1))
                return last

            gate0 = emit_conv_dt(0)
            fe["gate"] = gate0
            emit_conv_dt(1)
            emit_frontend()
            emit_conv_dt(2)
            emit_conv_dt(3)

            # ---------- epilogue: BCE = sum softplus(xcam) - sum xcam*y ----
            xcam_row = cp.tile([1, PADN], f32)
            nc.vector.memset(xcam_row[:], -30.0)  # softplus(pad) ~ 0
            for nh in range(NH):
                nc.vector.tensor_scalar(
                    xcam_row[0:1, nh * NF:(nh + 1) * NF], s_ps[nh][:],
                    sb_sb[:], None, op0=OP.add)

            xcp_ps = mps.tile([P, 7], f32, tag="mp")
            for a in range(7):
                nc.tensor.transpose(xcp_ps[:, a:a + 1],
                                    xcam_row[0:1, a * P:(a + 1) * P],
                                    ones11[:])
            xcamP = cp.tile([P, 7], f32)
            nc.vector.tensor_copy(xcamP[:], xcp_ps[:])

            # bce_sum = sum softplus(x) - inv*(S1 - cmin*S2) where
            # S1 = sum mask*cam*x, S2 = sum mask*x (all in [128,7] layout)
            expP = cp.tile([P, 7], f32)
            nc.scalar.activation(expP[:], xcamP[:], AF.Exp)
            spP = cp.tile([P, 7], f32)
            nc.scalar.activation(spP[:], expP[:], AF.Ln, bias=1.0)
            prod1 = cp.tile([P, 7], f32)
            nc.vector.tensor_mul(prod1[:], fe["ymP"][:], xcamP[:])
            prod2 = cp.tile([P, 7], f32)
            nc.vector.tensor_mul(prod2[:], fe["maskP"][:], xcamP[:])
            partial = cp.tile([P, 3], f32)
            nc.vector.reduce_sum(partial[:, 0:1], spP[:], axis=AX)
            nc.vector.reduce_sum(partial[:, 1:2], prod1[:], axis=AX)
            nc.vector.reduce_sum(partial[:, 2:3], prod2[:], axis=AX)

            dot_ps = mps.tile([1, 3], f32, tag="mp")
            nc.tensor.matmul(dot_ps[:], ones_col[:], partial[:],
                             start=True, stop=True)
            t1 = cp.tile([1, 1], f32)
            nc.vector.tensor_scalar(t1[:], dot_ps[0:1, 2:3], fe["cmin"][:],
                                    None, op0=OP.mult)
            t2 = cp.tile([1, 1], f32)
            nc.vector.tensor_tensor(t2[:], dot_ps[0:1, 1:2], t1[:],
                                    op=OP.subtract)
            t3 = cp.tile([1, 1], f32)
            nc.vector.tensor_scalar(t3[:], t2[:], fe["inv"][:], None,
                                    op0=OP.mult)
            final = cp.tile([1, 1], f32)
            nc.vector.tensor_tensor(final[:], dot_ps[0:1, 0:1], t3[:],
                                    op=OP.subtract)
            nc.sync.dma_start(out_d[:], final[:])

    nc.compile()
    return nc


def _prep_in_maps(x, x_fpv_pred, proj_weight, conv1_w, conv1_b, score_w,
                  score_b):
    import concourse.mybir as mybir
    bf16 = ml_dtypes.bfloat16
    fp8 = mybir.dt.np(mybir.dt.float8e4)

    # padded planes [B, CT, P, T, 9, 9] then 3 w-shifted 9x7 copies
    xr = np.asarray(x, np.float32).reshape(B, CT, P, T, H, W)
    xp9 = np.zeros((B, CT, P, T, 9, 9), np.float32)
    xp9[:, :, :, :, 1:8, 1:8] = xr
    xp9 = xp9.reshape(B, CTP, 2, P, T, 9, 9)
    # x3[b, ctp, two, p, s, t, h', w] = xp9[b, ctp, two, p, t, h', w+s]
    x3 = np.stack([xp9[..., s:s + 7] for s in range(3)], axis=4)
    # dims now (b, ctp, two, p, s, t, h', w) -> (b, ctp, p, two, s, t, h', w)
    xp8 = np.ascontiguousarray(
        x3.transpose(0, 1, 3, 2, 4, 5, 6, 7).reshape(B, CTP, P, XF)
    ).astype(fp8)

    w9 = np.asarray(conv1_w, np.float32).reshape(D, C, 9)
    # w8[dt, p, ((ctp*9 + tap)*2 + two)*P + q]
    #   = 64 * conv1_w[dt*P+q, (2*ctp+two)*P+p, tap]
    w8 = np.ascontiguousarray(
        (w9 * 64.0).reshape(DT, P, CTP, 2, P, 9).transpose(0, 4, 2, 5, 3, 1)
        .reshape(DT, P, CTP * 9 * 2 * P)).astype(fp8)

    proj_bf = np.asarray(proj_weight, np.float32).astype(bf16)
    cb = np.ascontiguousarray(
        np.asarray(conv1_b, np.float32).reshape(DT, P).T)
    sw = np.ascontiguousarray(
        np.asarray(score_w, np.float32).reshape(DT, P).T).astype(bf16)
    sb = np.asarray(score_b, np.float32).reshape(1, 1)
    xfp = np.asarray(x_fpv_pred, np.float32)

    in_maps = []
    for b in range(B):
        in_maps.append({
            "xp8": xp8[b],
            "w8": w8,
            "proj": proj_bf,
            "xfp": np.ascontiguousarray(xfp[b:b + 1]),
            "cb": cb,
            "sw": sw,
            "sb": sb,
        })
    return in_maps


def run(inputs, trace=False):
    """Build (cached), run on 8 cores, return (loss, BassKernelResults)."""
    from concourse.bass_utils import run_bass_kernel_spmd

    if "nc" not in _cache:
        _cache["nc"] = _build_nc()
    nc = _cache["nc"]
    in_maps = _prep_in_maps(**inputs)
    res = run_bass_kernel_spmd(nc, in_maps, core_ids=list(range(B)),
                               trace=trace)
    total = sum(float(np.asarray(res.results[b]["out"])[0, 0])
                for b in range(B))
    loss = np.float32(total / float(B * T * H * W))
    return loss, res


def kernel(**inputs):
    loss, _ = run(inputs, trace=False)
    return loss


# revision 14
# speedup vs baseline: 1.1582x; 1.1582x over previous
"""Trainium2 Bass kernel for nn_CAMLocalHead (CAM target + conv head + BCE).

Self-contained: takes FULL inputs, shards batch B=8 across 8 NeuronCores
(one sample per core), runs a Bass/Tile kernel per core, sums the per-core
partial BCE sums on host.

Device algorithm per core (one sample):
  - argmax class via one-hot (sigmoid is monotonic), selected proj row via
    PE matmuls, CAM = row @ x as fp8 DoubleRow matmuls (scale-invariant).
  - top-392-of-784 mask via rank trick: rank(v) = #{j: cam_j >= v} <= 392,
    computed with a PE broadcast + DVE is_ge accumulations (no sort).
  - Conv3d(2048->512, 1x3x3, pad 011) as 9 shifted fp8 DoubleRow matmuls
    accumulating in PSUM; x stored as 3 w-shifted padded copies so each
    tap reads contiguous 49-element runs per t-plane (no junk columns).
    Weights pre-scaled x64 into e4m3 range; un-scaled via ReLU activation
    scale=1/64. ReLU+bias fused on ACT; score conv = one more matmul per
    d-tile accumulating into a [1, 392] psum.
  - BCE sum = sum ln(1+e^x) - sum x*y  (softplus via Exp then Ln(1+e)).
"""
import sys

for _p in ("/opt/trn_rl_repo", "/opt/pypackages"):
    if _p not in sys.path:
        sys.path.append(_p)

import numpy as np
import ml_dtypes

# Problem dims (hardcoded per spec)
B, C, T, H, W = 8, 2048, 16, 7, 7
K, D = 400, 512
N_TOKEN = 392
P = 128
CT = C // P          # 16 c-tiles
CTP = CT // 2        # 8 c-tile pairs (DoubleRow)
DT = D // P          # 4 d-tiles
NH = 2               # spatial halves (t 0..7, 8..15)
TH = T // NH         # 8
NF = TH * H * W      # 392 positions per half
NPOS = T * H * W     # 784
PADN = 7 * P         # 896 (784 padded to 7 chunks of 128)
NEG = -1.0e30
SHW = 9 * 7          # 63: one w-shifted padded plane (9 rows x 7 cols)
SPT = T * SHW        # 1008: one shift-copy, all t
XF = 2 * 3 * SPT     # 6048: free size of one fp8 x pair-tile

_cache = {}


def _build_nc():
    import concourse.bacc as bacc
    import concourse.mybir as mybir
    from concourse import tile
    from concourse.tile_rust import add_dep_helper

    f32 = mybir.dt.float32
    bf16 = mybir.dt.bfloat16
    fp8 = mybir.dt.float8e4
    DR = mybir.MatmulPerfMode.DoubleRow
    AX = mybir.AxisListType.X
    OP = mybir.AluOpType
    AF = mybir.ActivationFunctionType

    nc = bacc.Bacc(trn_type="TRN2")

    w8_d = nc.dram_tensor("w8", [DT, P, CTP * 9 * 2 * P], fp8,
                          kind="ExternalInput")
    xp8_d = nc.dram_tensor("xp8", [CTP, P, XF], fp8, kind="ExternalInput")
    proj_d = nc.dram_tensor("proj", [K, C], bf16, kind="ExternalInput")
    xfp_d = nc.dram_tensor("xfp", [1, K], f32, kind="ExternalInput")
    cb_d = nc.dram_tensor("cb", [P, DT], f32, kind="ExternalInput")
    sw_d = nc.dram_tensor("sw", [P, DT], bf16, kind="ExternalInput")
    sb_d = nc.dram_tensor("sb", [1, 1], f32, kind="ExternalInput")
    out_d = nc.dram_tensor("out", [1, 1], f32, kind="ExternalOutput")

    with tile.TileContext(nc) as tc:
        with (
            tc.tile_pool(name="const", bufs=1) as cp,
            tc.tile_pool(name="wps_", bufs=4) as wp,
            tc.tile_pool(name="wpb_", bufs=2) as wpb,
            tc.tile_pool(name="rp", bufs=4) as rp,
            tc.tile_pool(name="cps", bufs=2, space="PSUM") as cps,
            tc.tile_pool(name="sps", bufs=1, space="PSUM") as sps,
            tc.tile_pool(name="mps", bufs=2, space="PSUM") as mps,
        ):
            # ---------- small constants (scalar HWDGE ring) ----------
            xfp = cp.tile([1, K], f32)
            nc.scalar.dma_start(xfp[:], xfp_d[:])
            proj_sb = cp.tile([P, 4 * C], bf16)
            for kc in range(4):
                kcnt = min(P, K - kc * P)
                nc.scalar.dma_start(
                    proj_sb[0:kcnt, kc * C:(kc + 1) * C],
                    proj_d[kc * P:kc * P + kcnt, :])
            cb_sb = cp.tile([P, DT], f32)
            nc.scalar.dma_start(cb_sb[:], cb_d[:])
            sw_sb = cp.tile([P, DT], bf16)
            nc.scalar.dma_start(sw_sb[:], sw_d[:])
            sb_sb = cp.tile([1, 1], f32)
            nc.scalar.dma_start(sb_sb[:], sb_d[:])

            ones11 = cp.tile([1, 1], f32)
            nc.vector.memset(ones11[:], 1.0)
            warm = cp.tile([1, 1], f32)
            nc.scalar.activation(warm[:], ones11[:], AF.Exp)
            nc.scalar.activation(warm[:], ones11[:], AF.Ln, bias=1.0)
            ones_row = cp.tile([1, P], f32)
            nc.vector.memset(ones_row[:], 1.0)
            ones_col = cp.tile([P, 1], f32)
            nc.vector.memset(ones_col[:], 1.0)

            xp8tiles = [cp.tile([P, XF], fp8, name=f"xp8_{i}")
                        for i in range(CTP)]

            def xp8view(ctp):
                # [p, two, s, t, f63]
                return xp8tiles[ctp][:].rearrange(
                    "p (two s t f) -> p two s t f", two=2, s=3, t=T, f=SHW)

            def conv_rhs(ctp, tap, nh):
                dh, dw = tap // 3, tap % 3
                v = xp8view(ctp)[:, :, dw, nh * TH:(nh + 1) * TH,
                                 dh * 7:dh * 7 + 49]
                return v  # [p, 2, TH, 49] -> free 784, halved by DoubleRow

            # ---------- CAM front-end (emitted between conv dt1 and dt2
            # so its DMA/DVE deps resolve while PE chews on conv) --------
            fe = {}

            def emit_frontend():
                # argmax class via one-hot (sigmoid monotonic)
                m = cp.tile([1, 1], f32)
                nc.vector.reduce_max(m[:], xfp[:], axis=AX)
                oh = cp.tile([1, 4 * P], f32)
                nc.vector.memset(oh[:], 0.0)
                nc.vector.tensor_scalar(oh[0:1, 0:K], xfp[:], m[:], None,
                                        op0=OP.is_equal)
                ohT_ps = mps.tile([P, 4], f32, tag="mp")
                for i in range(4):
                    nc.tensor.transpose(ohT_ps[:, i:i + 1],
                                        oh[0:1, i * P:(i + 1) * P],
                                        ones11[:])
                ohT = cp.tile([P, 4], bf16)
                nc.vector.tensor_copy(ohT[:], ohT_ps[:])

                # w_selT[c] = proj_weight[top_cls, c], [128, CT] c-tile cols
                wsel_ps = mps.tile([P, CT], f32, tag="mp")
                for ct in range(CT):
                    for kc in range(4):
                        kcnt = min(P, K - kc * P)
                        nc.tensor.matmul(
                            wsel_ps[:, ct:ct + 1],
                            proj_sb[0:kcnt,
                                    kc * C + ct * P:kc * C + (ct + 1) * P],
                            ohT[0:kcnt, kc:kc + 1],
                            start=(kc == 0), stop=(kc == 3))
                # wsel8[p, two*16 + ctp] = 64 * wsel[p, 2*ctp+two], fp8
                wsel8 = cp.tile([P, 32], fp8)
                wv_out = wsel8[:].rearrange("p (two q) -> p two q", two=2)
                wv_in = wsel_ps[:].rearrange("p (q two) -> p two q", two=2)
                nc.vector.tensor_scalar(wv_out[:, :, 0:CTP], wv_in, 64.0,
                                        None, op0=OP.mult)

                # cam[1, 784] = w_sel @ x (center tap), fp8 DoubleRow
                cam_ps = [mps.tile([1, NF], f32, tag="mp", name=f"cam_ps{_h}")
                          for _h in range(NH)]
                for nh in range(NH):
                    for ctp in range(CTP):
                        nc.tensor.matmul(
                            cam_ps[nh][:],
                            wv_out[:, :, ctp:ctp + 1],
                            conv_rhs(ctp, 4, nh),
                            start=(ctp == 0), stop=(ctp == CTP - 1),
                            perf_mode=DR)
                cam_row = cp.tile([1, PADN], f32)
                nc.vector.memset(cam_row[:], NEG)
                for nh in range(NH):
                    nc.vector.tensor_copy(
                        cam_row[0:1, nh * NF:(nh + 1) * NF], cam_ps[nh][:])

                # min/max for the (monotonic) normalization, done off the
                # PE critical path; ranks use RAW cam values.
                cmin = cp.tile([1, 1], f32)
                cmax = cp.tile([1, 1], f32)
                nc.vector.tensor_reduce(cmin[:], cam_row[0:1, 0:NPOS],
                                        axis=AX, op=OP.min)
                nc.vector.reduce_max(cmax[:], cam_row[0:1, 0:NPOS], axis=AX)
                rng_t = cp.tile([1, 1], f32)
                nc.vector.tensor_scalar(rng_t[:], cmax[:], cmin[:], None,
                                        op0=OP.subtract)
                inv = cp.tile([1, 1], f32)
                nc.vector.reciprocal(inv[:], rng_t[:])

                # broadcast raw cam across partitions: camB[128, 784]
                camB = cp.tile([P, NPOS], f32)
                for nh in range(NH):
                    cb_ps = mps.tile([P, NF], f32, tag="mp")
                    nc.tensor.matmul(cb_ps[:], ones_row[:],
                                     cam_row[0:1, nh * NF:(nh + 1) * NF],
                                     start=True, stop=True)
                    nc.vector.tensor_copy(
                        camB[:, nh * NF:(nh + 1) * NF], cb_ps[:])

                # raw cam in partition layout [128, 7]
                cnp_ps = mps.tile([P, 7], f32, tag="mp")
                for a in range(7):
                    nc.tensor.transpose(cnp_ps[:, a:a + 1],
                                        cam_row[0:1, a * P:(a + 1) * P],
                                        ones11[:])
                camP = cp.tile([P, 7], f32)
                nc.vector.tensor_copy(camP[:], cnp_ps[:])

                # rank[p,a] = #{j: cam[j] >= cam[p,a]}; top-392 = rank<=392
                ge = cp.tile([P, NPOS], f32)
                rank = cp.tile([P, 7], f32)
                for a in range(7):
                    nc.vector.tensor_scalar(ge[:], camB[:],
                                            camP[:, a:a + 1],
                                            None, op0=OP.is_ge, op1=OP.add,
                                            accum_out=rank[:, a:a + 1])
                maskP = cp.tile([P, 7], f32)
                nc.vector.tensor_scalar(maskP[:], rank[:], float(N_TOKEN),
                                        None, op0=OP.is_le)
                # y = mask * (cam - cmin) * inv; keep mask*cam (raw) and
                # mask separately -- min-max norm is folded into the final
                # scalar combine: sum(y*x) = inv*(S1 - cmin*S2).
                ymP = cp.tile([P, 7], f32)
                nc.vector.tensor_mul(ymP[:], maskP[:], camP[:])
                fe["ymP"] = ymP
                fe["maskP"] = maskP
                fe["cmin"] = cmin
                fe["inv"] = inv

            # ---------- conv main loop (fp8 DoubleRow) ----------
            s_ps = [sps.tile([1, NF], f32, tag=f"s{nh}", name=f"s_ps{nh}")
                    for nh in range(NH)]

            def emit_conv_dt(dt):
                ps = [cps.tile([P, NF], f32, tag=f"cv{nh}",
                               name=f"ps{dt}_{nh}")
                      for nh in range(NH)]
                if dt == 0:
                    wtile = None
                else:
                    wtile = wpb.tile([P, CTP * 9 * 2 * P], fp8, name="w_big",
                                     tag="w_big")
                    nc.sync.dma_start(wtile[:], w8_d[dt])
                for ctp in range(CTP):
                    if dt == 0:
                        w_ct = wp.tile([P, 9 * 2 * P], fp8, name="w_ct",
                                       tag="w_ct")
                        nc.sync.dma_start(
                            w_ct[:],
                            w8_d[dt][:, ctp * 9 * 2 * P:
                                     (ctp + 1) * 9 * 2 * P])
                        nc.sync.dma_start(xp8tiles[ctp][:], xp8_d[ctp])
                    for tap in range(9):
                        if dt == 0:
                            wsl = w_ct[:, tap * 2 * P:(tap + 1) * 2 * P]
                        else:
                            wsl = wtile[:, (ctp * 9 + tap) * 2 * P:
                                        (ctp * 9 + tap + 1) * 2 * P]
                        lhsT3 = wsl.rearrange("p (two q) -> p two q", two=2)
                        for nh in range(NH):
                            nc.tensor.matmul(
                                ps[nh][:], lhsT3, conv_rhs(ctp, tap, nh),
                                start=(ctp == 0 and tap == 0),
                                stop=(ctp == CTP - 1 and tap == 8),
                                perf_mode=DR)
                last = None
                for nh in range(NH):
                    relu_t = rp.tile([P, NF], bf16, name="relu_t")
                    nc.scalar.activation(relu_t[:], ps[nh][:], AF.Relu,
                                         bias=cb_sb[:, dt:dt + 1],
                                         scale=1.0 / 64.0)
                    last = nc.tensor.matmul(s_ps[nh][:], sw_sb[:, dt:dt + 1],
                                            relu_t[:],
                                            start=(dt == 0),
                                            stop=(dt == DT - 1))
                return last

            gate0 = emit_conv_dt(0)
            fe["gate"] = gate0
            emit_conv_dt(1)
            emit_frontend()
            emit_conv_dt(2)
            emit_conv_dt(3)

            # ---------- epilogue: BCE = sum softplus(xcam) - sum xcam*y ----
            xcam_row = cp.tile([1, PADN], f32)
            nc.vector.memset(xcam_row[:], -30.0)  # softplus(pad) ~ 0
            for nh in range(NH):
                nc.vector.tensor_scalar(
                    xcam_row[0:1, nh * NF:(nh + 1) * NF], s_ps[nh][:],
                    sb_sb[:], None, op0=OP.add)

            xcp_ps = mps.tile([P, 7], f32, tag="mp")
            for a in range(7):
                nc.tensor.transpose(xcp_ps[:, a:a + 1],
                                    xcam_row[0:1, a * P:(a + 1) * P],
                                    ones11[:])
            xcamP = cp.tile([P, 7], f32)
            nc.vector.tensor_copy(xcamP[:], xcp_ps[:])

            # bce_sum = sum softplus(x) - inv*(S1 - cmin*S2) where
            # S1 = sum mask*cam*x, S2 = sum mask*x (all in [128,7] layout)
            expP = cp.tile([P, 7], f32)
            nc.scalar.activation(expP[:], xcamP[:], AF.Exp)
            spP = cp.tile([P, 7], f32)
            nc.scalar.activation(spP[:], expP[:], AF.Ln, bias=1.0)
            prod1 = cp.tile([P, 7], f32)
            nc.vector.tensor_mul(prod1[:], fe["ymP"][:], xcamP[:])
            prod2 = cp.tile([P, 7], f32)
            nc.vector.tensor_mul(prod2[:], fe["maskP"][:], xcamP[:])
            partial = cp.tile([P, 3], f32)
            nc.vector.reduce_sum(partial[:, 0:1], spP[:], axis=AX)
            nc.vector.reduce_sum(partial[:, 1:2], prod1[:], axis=AX)
            nc.vector.reduce_sum(partial[:, 2:3], prod2[:], axis=AX)

            dot_ps = mps.tile([1, 3], f32, tag="mp")
            nc.tensor.matmul(dot_ps[:], ones_col[:], partial[:],
                             start=True, stop=True)
            t1 = cp.tile([1, 1], f32)
            nc.vector.tensor_scalar(t1[:], dot_ps[0:1, 2:3], fe["cmin"][:],
                                    None, op0=OP.mult)
            t2 = cp.tile([1, 1], f32)
            nc.vector.tensor_tensor(t2[:], dot_ps[0:1, 1:2], t1[:],
                                    op=OP.subtract)
            t3 = cp.tile([1, 1], f32)
            nc.vector.tensor_scalar(t3[:], t2[:], fe["inv"][:], None,
                                    op0=OP.mult)
            final = cp.tile([1, 1], f32)
            nc.vector.tensor_tensor(final[:], dot_ps[0:1, 0:1], t3[:],
                                    op=OP.subtract)
            nc.sync.dma_start(out_d[:], final[:])

    nc.compile()
    return nc


def _prep_in_maps(x, x_fpv_pred, proj_weight, conv1_w, conv1_b, score_w,
                  score_b):
    import concourse.mybir as mybir
    bf16 = ml_dtypes.bfloat16
    fp8 = mybir.dt.np(mybir.dt.float8e4)

    # padded planes [B, CT, P, T, 9, 9] then 3 w-shifted 9x7 copies
    xr = np.asarray(x, np.float32).reshape(B, CT, P, T, H, W)
    xp9 = np.zeros((B, CT, P, T, 9, 9), np.float32)
    xp9[:, :, :, :, 1:8, 1:8] = xr
    xp9 = xp9.reshape(B, CTP, 2, P, T, 9, 9)
    # x3[b, ctp, two, p, s, t, h', w] = xp9[b, ctp, two, p, t, h', w+s]
    x3 = np.stack([xp9[..., s:s + 7] for s in range(3)], axis=4)
    # dims now (b, ctp, two, p, s, t, h', w) -> (b, ctp, p, two, s, t, h', w)
    xp8 = np.ascontiguousarray(
        x3.transpose(0, 1, 3, 2, 4, 5, 6, 7).reshape(B, CTP, P, XF)
    ).astype(fp8)

    w9 = np.asarray(conv1_w, np.float32).reshape(D, C, 9)
    # w8[dt, p, ((ctp*9 + tap)*2 + two)*P + q]
    #   = 64 * conv1_w[dt*P+q, (2*ctp+two)*P+p, tap]
    w8 = np.ascontiguousarray(
        (w9 * 64.0).reshape(DT, P, CTP, 2, P, 9).transpose(0, 4, 2, 5, 3, 1)
        .reshape(DT, P, CTP * 9 * 2 * P)).astype(fp8)

    proj_bf = np.asarray(proj_weight, np.float32).astype(bf16)
    cb = np.ascontiguousarray(
        np.asarray(conv1_b, np.float32).reshape(DT, P).T)
    sw = np.ascontiguousarray(
        np.asarray(score_w, np.float32).reshape(DT, P).T).astype(bf16)
    sb = np.asarray(score_b, np.float32).reshape(1, 1)
    xfp = np.asarray(x_fpv_pred, np.float32)

    in_maps = []
    for b in range(B):
        in_maps.append({
            "xp8": xp8[b],
            "w8": w8,
            "proj": proj_bf,
            "xfp": np.ascontiguousarray(xfp[b:b + 1]),
            "cb": cb,
            "sw": sw,
            "sb": sb,
        })
    return in_maps


def run(inputs, trace=False):
    """Build (cached), run on 8 cores, return (loss, BassKernelResults)."""
    from concourse.bass_utils import run_bass_kernel_spmd

    if "nc" not in _cache:
        _cache["nc"] = _build_nc()
    nc = _cache["nc"]
    in_maps = _prep_in_maps(**inputs)
    res = run_bass_kernel_spmd(nc, in_maps, core_ids=list(range(B)),
                               trace=trace)
    total = sum(float(np.asarray(res.results[b]["out"])[0, 0])
                for b in range(B))
    loss = np.float32(total / float(B * T * H * W))
    return loss, res


def kernel(**inputs):
    loss, _ = run(inputs, trace=False)
    return loss


# revision 15
# speedup vs baseline: 1.1723x; 1.0121x over previous
"""Trainium2 Bass kernel for nn_CAMLocalHead (CAM target + conv head + BCE).

Self-contained: takes FULL inputs, shards batch B=8 across 8 NeuronCores
(one sample per core), runs a Bass/Tile kernel per core, sums the per-core
partial BCE sums on host.

Device algorithm per core (one sample):
  - argmax class via one-hot (sigmoid is monotonic), selected proj row via
    PE matmuls, CAM = row @ x as fp8 DoubleRow matmuls (scale-invariant).
  - top-392-of-784 mask via rank trick: rank(v) = #{j: cam_j >= v} <= 392,
    computed with a PE broadcast + DVE is_ge accumulations (no sort).
  - Conv3d(2048->512, 1x3x3, pad 011) as 9 shifted fp8 DoubleRow matmuls
    accumulating in PSUM; x stored as 3 w-shifted padded copies so each
    tap reads contiguous 49-element runs per t-plane (no junk columns).
    Weights pre-scaled x64 into e4m3 range; un-scaled via ReLU activation
    scale=1/64. ReLU+bias fused on ACT; score conv = one more matmul per
    d-tile accumulating into a [1, 392] psum.
  - BCE sum = sum ln(1+e^x) - sum x*y  (softplus via Exp then Ln(1+e)).
"""
import sys

for _p in ("/opt/trn_rl_repo", "/opt/pypackages"):
    if _p not in sys.path:
        sys.path.append(_p)

import numpy as np
import ml_dtypes

# Problem dims (hardcoded per spec)
B, C, T, H, W = 8, 2048, 16, 7, 7
K, D = 400, 512
N_TOKEN = 392
P = 128
CT = C // P          # 16 c-tiles
CTP = CT // 2        # 8 c-tile pairs (DoubleRow)
DT = D // P          # 4 d-tiles
NH = 2               # spatial halves (t 0..7, 8..15)
TH = T // NH         # 8
NF = TH * H * W      # 392 positions per half
NPOS = T * H * W     # 784
PADN = 7 * P         # 896 (784 padded to 7 chunks of 128)
NEG = -1.0e30
SHW = 9 * 7          # 63: one w-shifted padded plane (9 rows x 7 cols)
SPT = T * SHW        # 1008: one shift-copy, all t
XF = 2 * 3 * SPT     # 6048: free size of one fp8 x pair-tile

_cache = {}


def _build_nc():
    import concourse.bacc as bacc
    import concourse.mybir as mybir
    from concourse import tile
    from concourse.tile_rust import add_dep_helper

    f32 = mybir.dt.float32
    bf16 = mybir.dt.bfloat16
    fp8 = mybir.dt.float8e4
    DR = mybir.MatmulPerfMode.DoubleRow
    AX = mybir.AxisListType.X
    OP = mybir.AluOpType
    AF = mybir.ActivationFunctionType

    nc = bacc.Bacc(trn_type="TRN2")

    w8_d = nc.dram_tensor("w8", [DT, P, CTP * 9 * 2 * P], fp8,
                          kind="ExternalInput")
    xp8_d = nc.dram_tensor("xp8", [CTP, P, XF], fp8, kind="ExternalInput")
    proj_d = nc.dram_tensor("proj", [4 * P, C], fp8, kind="ExternalInput")
    xfp_d = nc.dram_tensor("xfp", [1, K], f32, kind="ExternalInput")
    cb_d = nc.dram_tensor("cb", [P, DT], f32, kind="ExternalInput")
    sw_d = nc.dram_tensor("sw", [P, DT], bf16, kind="ExternalInput")
    sb_d = nc.dram_tensor("sb", [1, 1], f32, kind="ExternalInput")
    out_d = nc.dram_tensor("out", [1, 1], f32, kind="ExternalOutput")

    with tile.TileContext(nc) as tc:
        with (
            tc.tile_pool(name="const", bufs=1) as cp,
            tc.tile_pool(name="wps_", bufs=4) as wp,
            tc.tile_pool(name="wpb_", bufs=2) as wpb,
            tc.tile_pool(name="rp", bufs=4) as rp,
            tc.tile_pool(name="cps", bufs=2, space="PSUM") as cps,
            tc.tile_pool(name="sps", bufs=1, space="PSUM") as sps,
            tc.tile_pool(name="mps", bufs=2, space="PSUM") as mps,
        ):
            # ---------- small constants (scalar HWDGE ring) ----------
            xfp = cp.tile([1, K], f32)
            nc.scalar.dma_start(xfp[:], xfp_d[:])
            proj_sb = cp.tile([P, 4 * C], fp8)
            for kc in range(4):
                nc.scalar.dma_start(
                    proj_sb[:, kc * C:(kc + 1) * C],
                    proj_d[kc * P:(kc + 1) * P, :])
            cb_sb = cp.tile([P, DT], f32)
            nc.scalar.dma_start(cb_sb[:], cb_d[:])
            sw_sb = cp.tile([P, DT], bf16)
            nc.scalar.dma_start(sw_sb[:], sw_d[:])
            sb_sb = cp.tile([1, 1], f32)
            nc.scalar.dma_start(sb_sb[:], sb_d[:])

            ones11 = cp.tile([1, 1], f32)
            nc.vector.memset(ones11[:], 1.0)
            warm = cp.tile([1, 1], f32)
            nc.scalar.activation(warm[:], ones11[:], AF.Exp)
            nc.scalar.activation(warm[:], ones11[:], AF.Ln, bias=1.0)
            ones_row = cp.tile([1, P], f32)
            nc.vector.memset(ones_row[:], 1.0)
            ones_col = cp.tile([P, 1], f32)
            nc.vector.memset(ones_col[:], 1.0)

            xp8tiles = [cp.tile([P, XF], fp8, name=f"xp8_{i}")
                        for i in range(CTP)]

            def xp8view(ctp):
                # [p, two, s, t, f63]
                return xp8tiles[ctp][:].rearrange(
                    "p (two s t f) -> p two s t f", two=2, s=3, t=T, f=SHW)

            def conv_rhs(ctp, tap, nh):
                dh, dw = tap // 3, tap % 3
                v = xp8view(ctp)[:, :, dw, nh * TH:(nh + 1) * TH,
                                 dh * 7:dh * 7 + 49]
                return v  # [p, 2, TH, 49] -> free 784, halved by DoubleRow

            # ---------- CAM front-end (emitted between conv dt1 and dt2
            # so its DMA/DVE deps resolve while PE chews on conv) --------
            fe = {}

            def emit_frontend():
                # argmax class via one-hot (sigmoid monotonic)
                m = cp.tile([1, 1], f32)
                nc.vector.reduce_max(m[:], xfp[:], axis=AX)
                oh = cp.tile([1, 4 * P], f32)
                nc.vector.memset(oh[:], 0.0)
                nc.vector.tensor_scalar(oh[0:1, 0:K], xfp[:], m[:], None,
                                        op0=OP.is_equal)
                ohT_ps = mps.tile([P, 4], f32, tag="mp")
                for i in range(4):
                    nc.tensor.transpose(ohT_ps[:, i:i + 1],
                                        oh[0:1, i * P:(i + 1) * P],
                                        ones11[:])
                # one-hot in DoubleRow pair layout: ohT2[p, two*16+kcp]
                ohT2 = cp.tile([P, 32], fp8)
                o2v = ohT2[:].rearrange("p (two q) -> p two q", two=2)
                nc.vector.tensor_copy(
                    o2v[:, :, 0:2],
                    ohT_ps[:].rearrange("p (kcp two) -> p two kcp", two=2))

                # w_selT[c] = 64*proj[top_cls, c] (proj pre-scaled), fp8 DR
                wsel_ps = mps.tile([P, CT], f32, tag="mp")
                for ct in range(CT):
                    for kcp in range(2):
                        lhsT = proj_sb[:, kcp * 2 * C + ct * P:].rearrange(
                            "p (two r) -> p two r", two=2)[:, :, 0:P]
                        nc.tensor.matmul(
                            wsel_ps[:, ct:ct + 1], lhsT,
                            o2v[:, :, kcp:kcp + 1],
                            start=(kcp == 0), stop=(kcp == 1),
                            perf_mode=DR)
                # wsel8[p, two*16 + ctp] = wsel_ps[p, 2*ctp+two], fp8
                wsel8 = cp.tile([P, 32], fp8)
                wv_out = wsel8[:].rearrange("p (two q) -> p two q", two=2)
                wv_in = wsel_ps[:].rearrange("p (q two) -> p two q", two=2)
                nc.vector.tensor_copy(wv_out[:, :, 0:CTP], wv_in)

                # cam[1, 784] = w_sel @ x (center tap), fp8 DoubleRow
                cam_ps = [mps.tile([1, NF], f32, tag="mp", name=f"cam_ps{_h}")
                          for _h in range(NH)]
                for nh in range(NH):
                    for ctp in range(CTP):
                        nc.tensor.matmul(
                            cam_ps[nh][:],
                            wv_out[:, :, ctp:ctp + 1],
                            conv_rhs(ctp, 4, nh),
                            start=(ctp == 0), stop=(ctp == CTP - 1),
                            perf_mode=DR)
                cam_row = cp.tile([1, PADN], f32)
                nc.vector.memset(cam_row[:], NEG)
                for nh in range(NH):
                    nc.vector.tensor_copy(
                        cam_row[0:1, nh * NF:(nh + 1) * NF], cam_ps[nh][:])

                # min/max for the (monotonic) normalization, done off the
                # PE critical path; ranks use RAW cam values.
                cmin = cp.tile([1, 1], f32)
                cmax = cp.tile([1, 1], f32)
                nc.vector.tensor_reduce(cmin[:], cam_row[0:1, 0:NPOS],
                                        axis=AX, op=OP.min)
                nc.vector.reduce_max(cmax[:], cam_row[0:1, 0:NPOS], axis=AX)
                rng_t = cp.tile([1, 1], f32)
                nc.vector.tensor_scalar(rng_t[:], cmax[:], cmin[:], None,
                                        op0=OP.subtract)
                inv = cp.tile([1, 1], f32)
                nc.vector.reciprocal(inv[:], rng_t[:])

                # broadcast raw cam across partitions: camB[128, 784]
                camB = cp.tile([P, NPOS], f32)
                for nh in range(NH):
                    cb_ps = mps.tile([P, NF], f32, tag="mp")
                    nc.tensor.matmul(cb_ps[:], ones_row[:],
                                     cam_row[0:1, nh * NF:(nh + 1) * NF],
                                     start=True, stop=True)
                    nc.vector.tensor_copy(
                        camB[:, nh * NF:(nh + 1) * NF], cb_ps[:])

                # raw cam in partition layout [128, 7]
                cnp_ps = mps.tile([P, 7], f32, tag="mp")
                for a in range(7):
                    nc.tensor.transpose(cnp_ps[:, a:a + 1],
                                        cam_row[0:1, a * P:(a + 1) * P],
                                        ones11[:])
                camP = cp.tile([P, 7], f32)
                nc.vector.tensor_copy(camP[:], cnp_ps[:])

                # rank[p,a] = #{j: cam[j] >= cam[p,a]}; top-392 = rank<=392
                ge = cp.tile([P, NPOS], f32)
                rank = cp.tile([P, 7], f32)
                for a in range(7):
                    nc.vector.tensor_scalar(ge[:], camB[:],
                                            camP[:, a:a + 1],
                                            None, op0=OP.is_ge, op1=OP.add,
                                            accum_out=rank[:, a:a + 1])
                maskP = cp.tile([P, 7], f32)
                nc.vector.tensor_scalar(maskP[:], rank[:], float(N_TOKEN),
                                        None, op0=OP.is_le)
                # y = mask * (cam - cmin) * inv; keep mask*cam (raw) and
                # mask separately -- min-max norm is folded into the final
                # scalar combine: sum(y*x) = inv*(S1 - cmin*S2).
                ymP = cp.tile([P, 7], f32)
                nc.vector.tensor_mul(ymP[:], maskP[:], camP[:])
                fe["ymP"] = ymP
                fe["maskP"] = maskP
                fe["cmin"] = cmin
                fe["inv"] = inv

            # ---------- conv main loop (fp8 DoubleRow) ----------
            s_ps = [sps.tile([1, NF], f32, tag=f"s{nh}", name=f"s_ps{nh}")
                    for nh in range(NH)]

            def emit_conv_dt(dt):
                ps = [cps.tile([P, NF], f32, tag=f"cv{nh}",
                               name=f"ps{dt}_{nh}")
                      for nh in range(NH)]
                if dt == 0:
                    wtile = None
                else:
                    wtile = wpb.tile([P, CTP * 9 * 2 * P], fp8, name="w_big",
                                     tag="w_big")
                    nc.sync.dma_start(wtile[:], w8_d[dt])
                for ctp in range(CTP):
                    if dt == 0:
                        w_ct = wp.tile([P, 9 * 2 * P], fp8, name="w_ct",
                                       tag="w_ct")
                        nc.sync.dma_start(
                            w_ct[:],
                            w8_d[dt][:, ctp * 9 * 2 * P:
                                     (ctp + 1) * 9 * 2 * P])
                        nc.sync.dma_start(xp8tiles[ctp][:], xp8_d[ctp])
                    for tap in range(9):
                        if dt == 0:
                            wsl = w_ct[:, tap * 2 * P:(tap + 1) * 2 * P]
                        else:
                            wsl = wtile[:, (ctp * 9 + tap) * 2 * P:
                                        (ctp * 9 + tap + 1) * 2 * P]
                        lhsT3 = wsl.rearrange("p (two q) -> p two q", two=2)
                        for nh in range(NH):
                            nc.tensor.matmul(
                                ps[nh][:], lhsT3, conv_rhs(ctp, tap, nh),
                                start=(ctp == 0 and tap == 0),
                                stop=(ctp == CTP - 1 and tap == 8),
                                perf_mode=DR)
                last = None
                for nh in range(NH):
                    relu_t = rp.tile([P, NF], bf16, name="relu_t")
                    nc.scalar.activation(relu_t[:], ps[nh][:], AF.Relu,
                                         bias=cb_sb[:, dt:dt + 1],
                                         scale=1.0 / 64.0)
                    last = nc.tensor.matmul(s_ps[nh][:], sw_sb[:, dt:dt + 1],
                                            relu_t[:],
                                            start=(dt == 0),
                                            stop=(dt == DT - 1))
                return last

            gate0 = emit_conv_dt(0)
            fe["gate"] = gate0
            emit_conv_dt(1)
            emit_frontend()
            emit_conv_dt(2)
            emit_conv_dt(3)

            # ---------- epilogue: BCE = sum softplus(xcam) - sum xcam*y ----
            xcam_row = cp.tile([1, PADN], f32)
            nc.vector.memset(xcam_row[:], -30.0)  # softplus(pad) ~ 0
            for nh in range(NH):
                nc.vector.tensor_scalar(
                    xcam_row[0:1, nh * NF:(nh + 1) * NF], s_ps[nh][:],
                    sb_sb[:], None, op0=OP.add)

            xcp_ps = mps.tile([P, 7], f32, tag="mp")
            for a in range(7):
                nc.tensor.transpose(xcp_ps[:, a:a + 1],
                                    xcam_row[0:1, a * P:(a + 1) * P],
                                    ones11[:])
            xcamP = cp.tile([P, 7], f32)
            nc.vector.tensor_copy(xcamP[:], xcp_ps[:])

            # bce_sum = sum softplus(x) - inv*(S1 - cmin*S2) where
            # S1 = sum mask*cam*x, S2 = sum mask*x (all in [128,7] layout)
            expP = cp.tile([P, 7], f32)
            nc.scalar.activation(expP[:], xcamP[:], AF.Exp)
            spP = cp.tile([P, 7], f32)
            nc.scalar.activation(spP[:], expP[:], AF.Ln, bias=1.0)
            prod1 = cp.tile([P, 7], f32)
            nc.vector.tensor_mul(prod1[:], fe["ymP"][:], xcamP[:])
            prod2 = cp.tile([P, 7], f32)
            nc.vector.tensor_mul(prod2[:], fe["maskP"][:], xcamP[:])
            partial = cp.tile([P, 3], f32)
            nc.vector.reduce_sum(partial[:, 0:1], spP[:], axis=AX)
            nc.vector.reduce_sum(partial[:, 1:2], prod1[:], axis=AX)
            nc.vector.reduce_sum(partial[:, 2:3], prod2[:], axis=AX)

            dot_ps = mps.tile([1, 3], f32, tag="mp")
            nc.tensor.matmul(dot_ps[:], ones_col[:], partial[:],
                             start=True, stop=True)
            t1 = cp.tile([1, 1], f32)
            nc.vector.tensor_scalar(t1[:], dot_ps[0:1, 2:3], fe["cmin"][:],
                                    None, op0=OP.mult)
            t2 = cp.tile([1, 1], f32)
            nc.vector.tensor_tensor(t2[:], dot_ps[0:1, 1:2], t1[:],
                                    op=OP.subtract)
            t3 = cp.tile([1, 1], f32)
            nc.vector.tensor_scalar(t3[:], t2[:], fe["inv"][:], None,
                                    op0=OP.mult)
            final = cp.tile([1, 1], f32)
            nc.vector.tensor_tensor(final[:], dot_ps[0:1, 0:1], t3[:],
                                    op=OP.subtract)
            nc.sync.dma_start(out_d[:], final[:])

    nc.compile()
    return nc


def _prep_in_maps(x, x_fpv_pred, proj_weight, conv1_w, conv1_b, score_w,
                  score_b):
    import concourse.mybir as mybir
    bf16 = ml_dtypes.bfloat16
    fp8 = mybir.dt.np(mybir.dt.float8e4)

    # padded planes [B, CT, P, T, 9, 9] then 3 w-shifted 9x7 copies
    xr = np.asarray(x, np.float32).reshape(B, CT, P, T, H, W)
    xp9 = np.zeros((B, CT, P, T, 9, 9), np.float32)
    xp9[:, :, :, :, 1:8, 1:8] = xr
    xp9 = xp9.reshape(B, CTP, 2, P, T, 9, 9)
    # x3[b, ctp, two, p, s, t, h', w] = xp9[b, ctp, two, p, t, h', w+s]
    x3 = np.stack([xp9[..., s:s + 7] for s in range(3)], axis=4)
    # dims now (b, ctp, two, p, s, t, h', w) -> (b, ctp, p, two, s, t, h', w)
    xp8 = np.ascontiguousarray(
        x3.transpose(0, 1, 3, 2, 4, 5, 6, 7).reshape(B, CTP, P, XF)
    ).astype(fp8)

    w9 = np.asarray(conv1_w, np.float32).reshape(D, C, 9)
    # w8[dt, p, ((ctp*9 + tap)*2 + two)*P + q]
    #   = 64 * conv1_w[dt*P+q, (2*ctp+two)*P+p, tap]
    w8 = np.ascontiguousarray(
        (w9 * 64.0).reshape(DT, P, CTP, 2, P, 9).transpose(0, 4, 2, 5, 3, 1)
        .reshape(DT, P, CTP * 9 * 2 * P)).astype(fp8)

    proj8 = np.zeros((4 * P, C), np.float32)
    proj8[:K] = np.asarray(proj_weight, np.float32) * 64.0
    proj8 = proj8.astype(fp8)
    cb = np.ascontiguousarray(
        np.asarray(conv1_b, np.float32).reshape(DT, P).T)
    sw = np.ascontiguousarray(
        np.asarray(score_w, np.float32).reshape(DT, P).T).astype(bf16)
    sb = np.asarray(score_b, np.float32).reshape(1, 1)
    xfp = np.asarray(x_fpv_pred, np.float32)

    in_maps = []
    for b in range(B):
        in_maps.append({
            "xp8": xp8[b],
            "w8": w8,
            "proj": proj8,
            "xfp": np.ascontiguousarray(xfp[b:b + 1]),
            "cb": cb,
            "sw": sw,
            "sb": sb,
        })
    return in_maps


def run(inputs, trace=False):
    """Build (cached), run on 8 cores, return (loss, BassKernelResults)."""
    from concourse.bass_utils import run_bass_kernel_spmd

    if "nc" not in _cache:
        _cache["nc"] = _build_nc()
    nc = _cache["nc"]
    in_maps = _prep_in_maps(**inputs)
    res = run_bass_kernel_spmd(nc, in_maps, core_ids=list(range(B)),
                               trace=trace)
    total = sum(float(np.asarray(res.results[b]["out"])[0, 0])
                for b in range(B))
    loss = np.float32(total / float(B * T * H * W))
    return loss, res


def kernel(**inputs):
    loss, _ = run(inputs, trace=False)
    return loss


# revision 16
# speedup vs baseline: 1.2084x; 1.0308x over previous
"""Trainium2 Bass kernel for nn_CAMLocalHead (CAM target + conv head + BCE).

Self-contained: takes FULL inputs, shards batch B=8 across 8 NeuronCores
(one sample per core), runs a Bass/Tile kernel per core, sums the per-core
partial BCE sums on host.

Device algorithm per core (one sample):
  - argmax class via one-hot (sigmoid is monotonic), selected proj row via
    PE matmuls, CAM = row @ x as fp8 DoubleRow matmuls (scale-invariant).
  - top-392-of-784 mask via rank trick: rank(v) = #{j: cam_j >= v} <= 392,
    computed with a PE broadcast + DVE is_ge accumulations (no sort).
  - Conv3d(2048->512, 1x3x3, pad 011) as 9 shifted fp8 DoubleRow matmuls
    accumulating in PSUM; x stored as 3 w-shifted padded copies so each
    tap reads contiguous 49-element runs per t-plane (no junk columns).
    Weights pre-scaled x64 into e4m3 range; un-scaled via ReLU activation
    scale=1/64. ReLU+bias fused on ACT; score conv = one more matmul per
    d-tile accumulating into a [1, 392] psum.
  - BCE sum = sum ln(1+e^x) - sum x*y  (softplus via Exp then Ln(1+e)).
"""
import sys

for _p in ("/opt/trn_rl_repo", "/opt/pypackages"):
    if _p not in sys.path:
        sys.path.append(_p)

import numpy as np
import ml_dtypes

# Problem dims (hardcoded per spec)
B, C, T, H, W = 8, 2048, 16, 7, 7
K, D = 400, 512
N_TOKEN = 392
P = 128
CT = C // P          # 16 c-tiles
CTP = CT // 2        # 8 c-tile pairs (DoubleRow)
DT = D // P          # 4 d-tiles
NH = 2               # spatial halves (t 0..7, 8..15)
TH = T // NH         # 8
NF = TH * H * W      # 392 positions per half
NPOS = T * H * W     # 784
PADN = 7 * P         # 896 (784 padded to 7 chunks of 128)
NEG = -1.0e30
SHW = 9 * 7          # 63: one w-shifted padded plane (9 rows x 7 cols)
SPT = T * SHW        # 1008: one shift-copy, all t
XF = 2 * 3 * SPT     # 6048: free size of one fp8 x pair-tile

_cache = {}


def _build_nc():
    import concourse.bacc as bacc
    import concourse.mybir as mybir
    from concourse import tile
    from concourse.tile_rust import add_dep_helper

    f32 = mybir.dt.float32
    bf16 = mybir.dt.bfloat16
    fp8 = mybir.dt.float8e4
    DR = mybir.MatmulPerfMode.DoubleRow
    AX = mybir.AxisListType.X
    OP = mybir.AluOpType
    AF = mybir.ActivationFunctionType

    nc = bacc.Bacc(trn_type="TRN2")

    w8_d = nc.dram_tensor("w8", [DT, P, CTP * 9 * 2 * P], fp8,
                          kind="ExternalInput")
    xp8_d = nc.dram_tensor("xp8", [CTP, P, XF], fp8, kind="ExternalInput")
    proj_d = nc.dram_tensor("proj", [4 * P, C], fp8, kind="ExternalInput")
    xfp_d = nc.dram_tensor("xfp", [1, K], f32, kind="ExternalInput")
    cb_d = nc.dram_tensor("cb", [P, DT], f32, kind="ExternalInput")
    sw_d = nc.dram_tensor("sw", [P, DT], bf16, kind="ExternalInput")
    sb_d = nc.dram_tensor("sb", [1, 1], f32, kind="ExternalInput")
    out_d = nc.dram_tensor("out", [1, 1], f32, kind="ExternalOutput")

    with tile.TileContext(nc) as tc:
        with (
            tc.tile_pool(name="const", bufs=1) as cp,
            tc.tile_pool(name="wps_", bufs=4) as wp,
            tc.tile_pool(name="wpb_", bufs=2) as wpb,
            tc.tile_pool(name="rp", bufs=4) as rp,
            tc.tile_pool(name="cps", bufs=2, space="PSUM") as cps,
            tc.tile_pool(name="sps", bufs=1, space="PSUM") as sps,
            tc.tile_pool(name="mps", bufs=2, space="PSUM") as mps,
        ):
            # ---------- small constants (scalar HWDGE ring) ----------
            xfp = cp.tile([1, K], f32)
            nc.scalar.dma_start(xfp[:], xfp_d[:])
            proj_sb = cp.tile([P, 4 * C], fp8)
            cb_sb = cp.tile([P, DT], f32)
            nc.scalar.dma_start(cb_sb[:], cb_d[:])
            sw_sb = cp.tile([P, DT], bf16)
            nc.scalar.dma_start(sw_sb[:], sw_d[:])
            sb_sb = cp.tile([1, 1], f32)
            nc.scalar.dma_start(sb_sb[:], sb_d[:])

            ones11 = cp.tile([1, 1], f32)
            nc.vector.memset(ones11[:], 1.0)
            warm = cp.tile([1, 1], f32)
            nc.scalar.activation(warm[:], ones11[:], AF.Exp)
            nc.scalar.activation(warm[:], ones11[:], AF.Ln, bias=1.0)
            ones_row = cp.tile([1, P], f32)
            nc.vector.memset(ones_row[:], 1.0)
            ones_col = cp.tile([P, 1], f32)
            nc.vector.memset(ones_col[:], 1.0)

            xp8tiles = [cp.tile([P, XF], fp8, name=f"xp8_{i}")
                        for i in range(CTP)]

            def xp8view(ctp):
                # [p, two, s, t, f63]
                return xp8tiles[ctp][:].rearrange(
                    "p (two s t f) -> p two s t f", two=2, s=3, t=T, f=SHW)

            def conv_rhs(ctp, tap, nh):
                dh, dw = tap // 3, tap % 3
                v = xp8view(ctp)[:, :, dw, nh * TH:(nh + 1) * TH,
                                 dh * 7:dh * 7 + 49]
                return v  # [p, 2, TH, 49] -> free 784, halved by DoubleRow

            # ---------- CAM front-end (emitted between conv dt1 and dt2
            # so its DMA/DVE deps resolve while PE chews on conv) --------
            fe = {}

            def emit_frontend():
                # argmax class via one-hot (sigmoid monotonic)
                m = cp.tile([1, 1], f32)
                nc.vector.reduce_max(m[:], xfp[:], axis=AX)
                oh = cp.tile([1, 4 * P], f32)
                nc.vector.memset(oh[:], 0.0)
                nc.vector.tensor_scalar(oh[0:1, 0:K], xfp[:], m[:], None,
                                        op0=OP.is_equal)
                ohT_ps = mps.tile([P, 4], f32, tag="mp")
                for i in range(4):
                    nc.tensor.transpose(ohT_ps[:, i:i + 1],
                                        oh[0:1, i * P:(i + 1) * P],
                                        ones11[:])
                # one-hot in DoubleRow pair layout: ohT2[p, two*16+kcp]
                ohT2 = cp.tile([P, 32], fp8)
                o2v = ohT2[:].rearrange("p (two q) -> p two q", two=2)
                nc.vector.tensor_copy(
                    o2v[:, :, 0:2],
                    ohT_ps[:].rearrange("p (kcp two) -> p two kcp", two=2))

                # w_selT[c] = 64*proj[top_cls, c] (proj pre-scaled), fp8 DR
                wsel_ps = mps.tile([P, CT], f32, tag="mp")
                for ct in range(CT):
                    for kcp in range(2):
                        lhsT = proj_sb[:, kcp * 2 * C + ct * P:].rearrange(
                            "p (two r) -> p two r", two=2)[:, :, 0:P]
                        nc.tensor.matmul(
                            wsel_ps[:, ct:ct + 1], lhsT,
                            o2v[:, :, kcp:kcp + 1],
                            start=(kcp == 0), stop=(kcp == 1),
                            perf_mode=DR)
                # wsel8[p, two*16 + ctp] = wsel_ps[p, 2*ctp+two], fp8
                wsel8 = cp.tile([P, 32], fp8)
                wv_out = wsel8[:].rearrange("p (two q) -> p two q", two=2)
                wv_in = wsel_ps[:].rearrange("p (q two) -> p two q", two=2)
                nc.vector.tensor_copy(wv_out[:, :, 0:CTP], wv_in)

                # cam[1, 784] = w_sel @ x (center tap), fp8 DoubleRow
                cam_ps = [mps.tile([1, NF], f32, tag="mp", name=f"cam_ps{_h}")
                          for _h in range(NH)]
                for nh in range(NH):
                    for ctp in range(CTP):
                        nc.tensor.matmul(
                            cam_ps[nh][:],
                            wv_out[:, :, ctp:ctp + 1],
                            conv_rhs(ctp, 4, nh),
                            start=(ctp == 0), stop=(ctp == CTP - 1),
                            perf_mode=DR)
                cam_row = cp.tile([1, PADN], f32)
                nc.vector.memset(cam_row[:], NEG)
                for nh in range(NH):
                    nc.vector.tensor_copy(
                        cam_row[0:1, nh * NF:(nh + 1) * NF], cam_ps[nh][:])

                # min/max for the (monotonic) normalization, done off the
                # PE critical path; ranks use RAW cam values.
                cmin = cp.tile([1, 1], f32)
                cmax = cp.tile([1, 1], f32)
                nc.vector.tensor_reduce(cmin[:], cam_row[0:1, 0:NPOS],
                                        axis=AX, op=OP.min)
                nc.vector.reduce_max(cmax[:], cam_row[0:1, 0:NPOS], axis=AX)
                rng_t = cp.tile([1, 1], f32)
                nc.vector.tensor_scalar(rng_t[:], cmax[:], cmin[:], None,
                                        op0=OP.subtract)
                inv = cp.tile([1, 1], f32)
                nc.vector.reciprocal(inv[:], rng_t[:])

                # broadcast raw cam across partitions: camB[128, 784]
                camB = cp.tile([P, NPOS], f32)
                for nh in range(NH):
                    cb_ps = mps.tile([P, NF], f32, tag="mp")
                    nc.tensor.matmul(cb_ps[:], ones_row[:],
                                     cam_row[0:1, nh * NF:(nh + 1) * NF],
                                     start=True, stop=True)
                    nc.vector.tensor_copy(
                        camB[:, nh * NF:(nh + 1) * NF], cb_ps[:])

                # raw cam in partition layout [128, 7]
                cnp_ps = mps.tile([P, 7], f32, tag="mp")
                for a in range(7):
                    nc.tensor.transpose(cnp_ps[:, a:a + 1],
                                        cam_row[0:1, a * P:(a + 1) * P],
                                        ones11[:])
                camP = cp.tile([P, 7], f32)
                nc.vector.tensor_copy(camP[:], cnp_ps[:])

                # rank[p,a] = #{j: cam[j] >= cam[p,a]}; top-392 = rank<=392
                ge = cp.tile([P, NPOS], f32)
                rank = cp.tile([P, 7], f32)
                for a in range(7):
                    nc.vector.tensor_scalar(ge[:], camB[:],
                                            camP[:, a:a + 1],
                                            None, op0=OP.is_ge, op1=OP.add,
                                            accum_out=rank[:, a:a + 1])
                maskP = cp.tile([P, 7], f32)
                nc.vector.tensor_scalar(maskP[:], rank[:], float(N_TOKEN),
                                        None, op0=OP.is_le)
                # y = mask * (cam - cmin) * inv; keep mask*cam (raw) and
                # mask separately -- min-max norm is folded into the final
                # scalar combine: sum(y*x) = inv*(S1 - cmin*S2).
                ymP = cp.tile([P, 7], f32)
                nc.vector.tensor_mul(ymP[:], maskP[:], camP[:])
                fe["ymP"] = ymP
                fe["maskP"] = maskP
                fe["cmin"] = cmin
                fe["inv"] = inv

            # ---------- conv main loop (fp8 DoubleRow) ----------
            s_ps = [sps.tile([1, NF], f32, tag=f"s{nh}", name=f"s_ps{nh}")
                    for nh in range(NH)]

            def emit_conv_dt(dt):
                ps = [cps.tile([P, NF], f32, tag=f"cv{nh}",
                               name=f"ps{dt}_{nh}")
                      for nh in range(NH)]
                if dt == 0:
                    wtile = None
                else:
                    wtile = wpb.tile([P, CTP * 9 * 2 * P], fp8, name="w_big",
                                     tag="w_big")
                    nc.sync.dma_start(wtile[:], w8_d[dt])
                for ctp in range(CTP):
                    if dt == 0:
                        w_ct = wp.tile([P, 9 * 2 * P], fp8, name="w_ct",
                                       tag="w_ct")
                        nc.sync.dma_start(
                            w_ct[:],
                            w8_d[dt][:, ctp * 9 * 2 * P:
                                     (ctp + 1) * 9 * 2 * P])
                        nc.sync.dma_start(xp8tiles[ctp][:], xp8_d[ctp])
                        if 2 <= ctp < 6:
                            kc = ctp - 2
                            nc.sync.dma_start(
                                proj_sb[:, kc * C:(kc + 1) * C],
                                proj_d[kc * P:(kc + 1) * P, :])
                    for tap in range(9):
                        if dt == 0:
                            wsl = w_ct[:, tap * 2 * P:(tap + 1) * 2 * P]
                        else:
                            wsl = wtile[:, (ctp * 9 + tap) * 2 * P:
                                        (ctp * 9 + tap + 1) * 2 * P]
                        lhsT3 = wsl.rearrange("p (two q) -> p two q", two=2)
                        for nh in range(NH):
                            nc.tensor.matmul(
                                ps[nh][:], lhsT3, conv_rhs(ctp, tap, nh),
                                start=(ctp == 0 and tap == 0),
                                stop=(ctp == CTP - 1 and tap == 8),
                                perf_mode=DR)
                last = None
                for nh in range(NH):
                    relu_t = rp.tile([P, NF], bf16, name="relu_t")
                    nc.scalar.activation(relu_t[:], ps[nh][:], AF.Relu,
                                         bias=cb_sb[:, dt:dt + 1],
                                         scale=1.0 / 64.0)
                    last = nc.tensor.matmul(s_ps[nh][:], sw_sb[:, dt:dt + 1],
                                            relu_t[:],
                                            start=(dt == 0),
                                            stop=(dt == DT - 1))
                return last

            gate0 = emit_conv_dt(0)
            fe["gate"] = gate0
            emit_conv_dt(1)
            emit_frontend()
            emit_conv_dt(2)
            emit_conv_dt(3)

            # ---------- epilogue: BCE = sum softplus(xcam) - sum xcam*y ----
            xcam_row = cp.tile([1, PADN], f32)
            nc.vector.memset(xcam_row[:], -30.0)  # softplus(pad) ~ 0
            for nh in range(NH):
                nc.vector.tensor_scalar(
                    xcam_row[0:1, nh * NF:(nh + 1) * NF], s_ps[nh][:],
                    sb_sb[:], None, op0=OP.add)

            xcp_ps = mps.tile([P, 7], f32, tag="mp")
            for a in range(7):
                nc.tensor.transpose(xcp_ps[:, a:a + 1],
                                    xcam_row[0:1, a * P:(a + 1) * P],
                                    ones11[:])
            xcamP = cp.tile([P, 7], f32)
            nc.vector.tensor_copy(xcamP[:], xcp_ps[:])

            # bce_sum = sum softplus(x) - inv*(S1 - cmin*S2) where
            # S1 = sum mask*cam*x, S2 = sum mask*x (all in [128,7] layout)
            expP = cp.tile([P, 7], f32)
            nc.scalar.activation(expP[:], xcamP[:], AF.Exp)
            spP = cp.tile([P, 7], f32)
            nc.scalar.activation(spP[:], expP[:], AF.Ln, bias=1.0)
            prod1 = cp.tile([P, 7], f32)
            nc.vector.tensor_mul(prod1[:], fe["ymP"][:], xcamP[:])
            prod2 = cp.tile([P, 7], f32)
            nc.vector.tensor_mul(prod2[:], fe["maskP"][:], xcamP[:])
            partial = cp.tile([P, 3], f32)
            nc.vector.reduce_sum(partial[:, 0:1], spP[:], axis=AX)
            nc.vector.reduce_sum(partial[:, 1:2], prod1[:], axis=AX)
            nc.vector.reduce_sum(partial[:, 2:3], prod2[:], axis=AX)

            dot_ps = mps.tile([1, 3], f32, tag="mp")
            nc.tensor.matmul(dot_ps[:], ones_col[:], partial[:],
                             start=True, stop=True)
            t1 = cp.tile([1, 1], f32)
            nc.vector.tensor_scalar(t1[:], dot_ps[0:1, 2:3], fe["cmin"][:],
                                    None, op0=OP.mult)
            t2 = cp.tile([1, 1], f32)
            nc.vector.tensor_tensor(t2[:], dot_ps[0:1, 1:2], t1[:],
                                    op=OP.subtract)
            t3 = cp.tile([1, 1], f32)
            nc.vector.tensor_scalar(t3[:], t2[:], fe["inv"][:], None,
                                    op0=OP.mult)
            final = cp.tile([1, 1], f32)
            nc.vector.tensor_tensor(final[:], dot_ps[0:1, 0:1], t3[:],
                                    op=OP.subtract)
            nc.sync.dma_start(out_d[:], final[:])

    nc.compile()
    return nc


def _prep_in_maps(x, x_fpv_pred, proj_weight, conv1_w, conv1_b, score_w,
                  score_b):
    import concourse.mybir as mybir
    bf16 = ml_dtypes.bfloat16
    fp8 = mybir.dt.np(mybir.dt.float8e4)

    # padded planes [B, CT, P, T, 9, 9] then 3 w-shifted 9x7 copies
    xr = np.asarray(x, np.float32).reshape(B, CT, P, T, H, W)
    xp9 = np.zeros((B, CT, P, T, 9, 9), np.float32)
    xp9[:, :, :, :, 1:8, 1:8] = xr
    xp9 = xp9.reshape(B, CTP, 2, P, T, 9, 9)
    # x3[b, ctp, two, p, s, t, h', w] = xp9[b, ctp, two, p, t, h', w+s]
    x3 = np.stack([xp9[..., s:s + 7] for s in range(3)], axis=4)
    # dims now (b, ctp, two, p, s, t, h', w) -> (b, ctp, p, two, s, t, h', w)
    xp8 = np.ascontiguousarray(
        x3.transpose(0, 1, 3, 2, 4, 5, 6, 7).reshape(B, CTP, P, XF)
    ).astype(fp8)

    w9 = np.asarray(conv1_w, np.float32).reshape(D, C, 9)
    # w8[dt, p, ((ctp*9 + tap)*2 + two)*P + q]
    #   = 64 * conv1_w[dt*P+q, (2*ctp+two)*P+p, tap]
    w8 = np.ascontiguousarray(
        (w9 * 64.0).reshape(DT, P, CTP, 2, P, 9).transpose(0, 4, 2, 5, 3, 1)
        .reshape(DT, P, CTP * 9 * 2 * P)).astype(fp8)

    proj8 = np.zeros((4 * P, C), np.float32)
    proj8[:K] = np.asarray(proj_weight, np.float32) * 64.0
    proj8 = proj8.astype(fp8)
    cb = np.ascontiguousarray(
        np.asarray(conv1_b, np.float32).reshape(DT, P).T)
    sw = np.ascontiguousarray(
        np.asarray(score_w, np.float32).reshape(DT, P).T).astype(bf16)
    sb = np.asarray(score_b, np.float32).reshape(1, 1)
    xfp = np.asarray(x_fpv_pred, np.float32)

    in_maps = []
    for b in range(B):
        in_maps.append({
            "xp8": xp8[b],
            "w8": w8,
            "proj": proj8,
            "xfp": np.ascontiguousarray(xfp[b:b + 1]),
            "cb": cb,
            "sw": sw,
            "sb": sb,
        })
    return in_maps


def run(inputs, trace=False):
    """Build (cached), run on 8 cores, return (loss, BassKernelResults)."""
    from concourse.bass_utils import run_bass_kernel_spmd

    if "nc" not in _cache:
        _cache["nc"] = _build_nc()
    nc = _cache["nc"]
    in_maps = _prep_in_maps(**inputs)
    res = run_bass_kernel_spmd(nc, in_maps, core_ids=list(range(B)),
                               trace=trace)
    total = sum(float(np.asarray(res.results[b]["out"])[0, 0])
                for b in range(B))
    loss = np.float32(total / float(B * T * H * W))
    return loss, res


def kernel(**inputs):
    loss, _ = run(inputs, trace=False)
    return loss
